# revision 1
# baseline (speedup 1.0000x reference)
"""CoverageAwareAttention on 8 TRN2 NeuronCores — v3 (restructured dataflow).

Key structure vs v2:
- coverage gate applied inside the exp activation via per-partition `scale`
  (S^T layout puts k on partitions) -> no gate-fold matmuls/DVE muls at all
- gate head-weights z computed TRANSPOSED ([k,3] per k-chunk) by tiny F=3
  matmuls; bg2 folded in via a ones-row matmul
- AV computed in O-layout (out[q, hd], F=65 incl denominator ones-column):
  half the PE rows of the O^T form; softmax denominators ride along as col 64
- normalization = per-partition reciprocal + tensor_scalar_mul (gpsimd)
- output projection computed transposed (out^T[d, tok]); host transposes
- one-head-lag pipeline: AV/normalize/transpose/outproj of head h interleave
  as PE filler into the ACT-bound exp phase of head h+1
"""

import numpy as np
import ml_dtypes

B, N, D = 2, 2048, 768
H, HD = 12, 64
GATE_HID = D // 4
SCALE = HD ** -0.5
NCORES = 8
EXP_BIAS = -3.0

_CACHE = {}


def _build(dbg=False):
    import concourse.tile as tile
    import concourse.mybir as mybir
    from concourse import bacc
    from concourse.masks import make_identity

    f32 = mybir.dt.float32
    f32r = mybir.dt.float32r
    f16 = mybir.dt.float16
    bf16 = mybir.dt.bfloat16
    AF = mybir.ActivationFunctionType

    nc = bacc.Bacc("TRN2", target_bir_lowering=False, debug=False,
                   num_devices=NCORES)

    xT = nc.dram_tensor("xT", [D, N], bf16, kind="ExternalInput").ap()
    cov = nc.dram_tensor("cov", [1, N], f32, kind="ExternalInput").ap()
    wqk = nc.dram_tensor("wqk", [D, 384], bf16, kind="ExternalInput").ap()
    wv = nc.dram_tensor("wv", [D, 192], bf16, kind="ExternalInput").ap()
    woA = nc.dram_tensor("woA", [128, D], f16, kind="ExternalInput").ap()
    woB = nc.dram_tensor("woB", [64, D], f16, kind="ExternalInput").ap()
    wg1 = nc.dram_tensor("wg1", [1, GATE_HID], f32, kind="ExternalInput").ap()
    bg1 = nc.dram_tensor("bg1", [GATE_HID, 1], f32, kind="ExternalInput").ap()
    wg2a = nc.dram_tensor("wg2a", [128, 3], bf16, kind="ExternalInput").ap()
    wg2b = nc.dram_tensor("wg2b", [64, 3], bf16, kind="ExternalInput").ap()
    bg2r = nc.dram_tensor("bg2r", [1, 3], bf16, kind="ExternalInput").ap()
    outp = nc.dram_tensor("outp", [D, N], f32, kind="ExternalOutput").ap()
    if dbg:
        gT_o = nc.dram_tensor("gT_o", [128, 48], f32, kind="ExternalOutput").ap()
        qkT_o = nc.dram_tensor("qkT_o", [128, 2 * N], f32, kind="ExternalOutput").ap()
        qk2_o = nc.dram_tensor("qk2_o", [64, 2 * N], f32, kind="ExternalOutput").ap()
        vaug_o = nc.dram_tensor("vaug_o", [128, 16 * 195], f32, kind="ExternalOutput").ap()
        OTA_o = nc.dram_tensor("OTA_o", [128, N], f32, kind="ExternalOutput").ap()
        OTB_o = nc.dram_tensor("OTB_o", [64, N], f32, kind="ExternalOutput").ap()

    with tile.TileContext(nc) as tc:
        with tc.tile_pool(name="const", bufs=1) as cp, \
             tc.tile_pool(name="onrm", bufs=10) as onp_, \
             tc.tile_pool(name="recp", bufs=4) as recp, \
             tc.tile_pool(name="stg", bufs=3) as stg, \
             tc.tile_pool(name="ptp", bufs=36) as ptp, \
             tc.tile_pool(name="psS", bufs=3, space="PSUM") as psS, \
             tc.tile_pool(name="psD", bufs=2, space="PSUM") as psD:

            # ------------- input DMAs ---------------------------------------
            cov_sb = cp.tile([1, N], f32r, tag="cov")
            nc.sync.dma_start(cov_sb[:], cov.bitcast(f32r))
            wg1_sb = cp.tile([1, GATE_HID], f32r, tag="wg1")
            nc.sync.dma_start(wg1_sb[:], wg1.bitcast(f32r))
            bg1a = cp.tile([128, 1], f32, tag="bg1a")
            nc.sync.dma_start(bg1a[:], bg1[0:128, :])
            bg1b = cp.tile([64, 1], f32, tag="bg1b")
            nc.sync.dma_start(bg1b[:], bg1[128:192, :])
            wg2a_sb = cp.tile([128, 3], bf16, tag="wg2a")
            nc.sync.dma_start(wg2a_sb[:], wg2a)
            wg2b_sb = cp.tile([64, 3], bf16, tag="wg2b")
            nc.sync.dma_start(wg2b_sb[:], wg2b)
            bg2_sb = cp.tile([1, 3], bf16, tag="bg2")
            nc.sync.dma_start(bg2_sb[:], bg2r)

            xT_sb = cp.tile([128, 6, N], bf16, tag="xT")
            xT_r = xT.rearrange("(c p) n -> p c n", p=128)
            for s in range(4):
                for c in range(6):
                    nc.sync.dma_start(xT_sb[:, c, s * 512:(s + 1) * 512],
                                      xT_r[:, c, s * 512:(s + 1) * 512])
            wqk_sb = cp.tile([128, 6, 384], bf16, tag="wqk")
            wqk_r = wqk.rearrange("(c p) f -> p c f", p=128)
            for c in range(6):
                nc.sync.dma_start(wqk_sb[:, c, :], wqk_r[:, c, :])
            wv_sb = cp.tile([128, 6, 192], bf16, tag="wv")
            wv_r = wv.rearrange("(c p) f -> p c f", p=128)
            for c in range(6):
                nc.sync.dma_start(wv_sb[:, c, :], wv_r[:, c, :])
            woA_sb = cp.tile([128, D], f16, tag="woA")
            nc.sync.dma_start(woA_sb[:], woA)
            woB_sb = cp.tile([64, D], f16, tag="woB")
            nc.sync.dma_start(woB_sb[:], woB)

            # ------------- const tiles --------------------------------------
            identF = cp.tile([128, 128], f16, tag="identF")
            make_identity(nc, identF[:])
            ebias = cp.tile([128, 1], f32, tag="ebias")
            nc.vector.memset(ebias[:], EXP_BIAS)
            zbias = cp.tile([128, 1], f32, tag="zbias")
            nc.vector.memset(zbias[:], 0.0)
            ones1 = cp.tile([1, 128], bf16, tag="ones1")
            nc.vector.memset(ones1[:], 1.0)
            vaug = cp.tile([128, 16, 195], f16, tag="vaug")
            nc.vector.memset(
                vaug.rearrange("p t (h c) -> p t h c", c=65)[:, :, :, 64], 1.0)

            qkT = cp.tile([128, 2, N], f16, tag="qkT")
            qk2 = cp.tile([64, 2, N], f16, tag="qk2")
            gT = cp.tile([128, 48], f32, tag="gT")
            ga_sb = cp.tile([128, 4, 512], bf16, tag="ga")
            gb_sb = cp.tile([64, 4, 512], bf16, tag="gb")
            OTA = cp.tile([128, N], f16, tag="OTA")
            OTB = cp.tile([64, N], f16, tag="OTB")

            # ------------- gate: hidden = silu(cov @ Wg1 + bg1) -------------
            for half in range(2):
                pa = psS.tile([128, 1024], f32, tag="S")
                for si in range(2):
                    sl = slice((2 * half + si) * 512, (2 * half + si + 1) * 512)
                    nc.tensor.matmul(pa[:, si * 512:(si + 1) * 512],
                                     wg1_sb[0:1, 0:128], cov_sb[0:1, sl],
                                     start=True, stop=True, skip_group_check=True)
                nc.scalar.activation(ga_sb[:, 2 * half:2 * half + 2, :], pa[:],
                                     AF.Silu, bias=bg1a[:, 0:1])
            for half in range(2):
                pb = psS.tile([64, 1024], f32, tag="S")
                for si in range(2):
                    sl = slice((2 * half + si) * 512, (2 * half + si + 1) * 512)
                    nc.tensor.matmul(pb[:, si * 512:(si + 1) * 512],
                                     wg1_sb[0:1, 128:192], cov_sb[0:1, sl],
                                     start=True, stop=True, skip_group_check=True)
                nc.scalar.activation(gb_sb[:, 2 * half:2 * half + 2, :], pb[:],
                                     AF.Silu, bias=bg1b[:, 0:1])

            # ------------- gate: zT[k, h] = hidden^T @ Wg2 + bg2, sigmoid ---
            zT_ps = psS.tile([128, 48], f32, tag="S")
            for kt in range(16):
                si, j = kt // 4, kt % 4
                out = zT_ps[:, 3 * kt:3 * kt + 3]
                nc.tensor.matmul(out, ga_sb[:, si, j * 128:(j + 1) * 128],
                                 wg2a_sb[:], start=True, stop=False,
                                 skip_group_check=True)
                nc.tensor.matmul(out, gb_sb[:, si, j * 128:(j + 1) * 128],
                                 wg2b_sb[:], start=False, stop=False,
                                 skip_group_check=True)
                nc.tensor.matmul(out, ones1[0:1, :], bg2_sb[0:1, :],
                                 start=False, stop=True,
                                 skip_group_check=True)
            nc.scalar.activation(gT[:], zT_ps[:], AF.Sigmoid, bias=zbias[:, 0:1])

            # ------------- qk projection (cg 0,1 now; cg2 as filler) --------
            def qk_chunk(cg, s, pool=None):
                ps = (pool or psS).tile([128, 512], f32,
                                        tag="S" if pool is None else "D")
                for c in range(6):
                    nc.tensor.matmul(ps[:],
                                     wqk_sb[:, c, cg * 128:(cg + 1) * 128],
                                     xT_sb[:, c, s * 512:(s + 1) * 512],
                                     start=(c == 0), stop=(c == 5),
                                     skip_group_check=True)
                nc.vector.tensor_copy(qkT[:, cg, s * 512:(s + 1) * 512], ps[:])

            for cg in (0, 1):
                for s in range(4):
                    qk_chunk(cg, s)

            # ------------- filler helpers -----------------------------------
            def vproj_chunk(t):
                ps = psD.tile([128, 192], f32, tag="D")
                for c in range(6):
                    nc.tensor.matmul(ps[:], xT_sb[:, c, t * 128:(t + 1) * 128],
                                     wv_sb[:, c, :], start=(c == 0),
                                     stop=(c == 5), skip_group_check=True)
                nc.vector.tensor_copy(
                    vaug.rearrange("p t (h c) -> p t h c", c=65)
                    [:, t, :, 0:64],
                    ps.rearrange("p (h c) -> p h c", c=64)[:])

            def qk2_chunk(s):
                ps = psD.tile([128, 512], f32, tag="D")
                for c in range(6):
                    nc.tensor.matmul(ps[:], wqk_sb[:, c, 256:384],
                                     xT_sb[:, c, s * 512:(s + 1) * 512],
                                     start=(c == 0), stop=(c == 5),
                                     skip_group_check=True)
                sl = slice(s * 512, (s + 1) * 512)
                nc.vector.tensor_copy(qk2[:, 0, sl], ps[0:64, :])
                nc.vector.tensor_copy(qk2[:, 1, sl], ps[64:128, :])

            def transpose_qt(qp, qt, onorm):
                qg = qp * 8 + qt
                T = psD.tile([128, 256], f16, tag="D")
                nc.tensor.transpose(T[:, 0:128], onorm[:, 0:128], identF[:])
                nc.tensor.transpose(T[0:64, 128:256], onorm[:, 128:192],
                                    identF[:])
                nc.vector.tensor_copy(OTA[:, qg * 128:(qg + 1) * 128],
                                      T[:, 0:128])
                nc.vector.tensor_copy(OTB[:, qg * 128:(qg + 1) * 128],
                                      T[0:64, 128:256])

            def outproj_chunk(qp, tb, dg):
                t0 = qp * 1024 + tb * 512
                ps = psD.tile([128, 512], f32, tag="D")
                nc.tensor.matmul(ps[:], woA_sb[:, dg * 128:(dg + 1) * 128],
                                 OTA[:, t0:t0 + 512], start=True, stop=False,
                                 skip_group_check=True)
                nc.tensor.matmul(ps[:], woB_sb[:, dg * 128:(dg + 1) * 128],
                                 OTB[:, t0:t0 + 512], start=False, stop=True,
                                 skip_group_check=True)
                ot = stg.tile([128, 512], f32, tag="ot")
                nc.vector.tensor_copy(ot[:], ps[:])
                nc.sync.dma_start(outp[dg * 128:(dg + 1) * 128, t0:t0 + 512],
                                  ot[:])

            # ------------- attention ---------------------------------------
            # S^T strip matmul bases per head: (lhsT base, rhs base, cg pair)
            def s_strip(qp, h, kt):
                S_ps = psS.tile([128, 1024], f32, tag="S")
                ksl = slice(kt * 128, (kt + 1) * 128)
                for qs in range(2):
                    q0 = qp * 1024 + qs * 512
                    out = S_ps[:, qs * 512:(qs + 1) * 512]
                    if h == 0:
                        nc.tensor.matmul(out, qkT[0:64, 1, ksl],
                                         qkT[0:64, 0, q0:q0 + 512],
                                         start=True, stop=True,
                                         skip_group_check=True)
                    elif h == 1:
                        nc.tensor.matmul(out, qkT[64:128, 1, ksl],
                                         qkT[64:128, 0, q0:q0 + 512],
                                         start=True, stop=True,
                                         skip_group_check=True)
                    else:
                        nc.tensor.matmul(out, qk2[:, 1, ksl],
                                         qk2[:, 0, q0:q0 + 512],
                                         start=True, stop=True,
                                         skip_group_check=True)
                PT = ptp.tile([128, 1024], f16, tag="PT")
                nc.scalar.activation(PT[:], S_ps[:], AF.Exp,
                                     bias=ebias[:, 0:1],
                                     scale=gT[:, 3 * kt + h:3 * kt + h + 1])
                return PT

            def av_qtile(pts, h, qt, onorms):
                og = psD.tile([128, 65], f32, tag="D",
                              padded_shape=[128, 512])
                for kt in range(16):
                    nc.tensor.matmul(
                        og[:], pts[kt][:, qt * 128:(qt + 1) * 128],
                        vaug[:, kt, h * 65:h * 65 + 65],
                        start=(kt == 0), stop=(kt == 15),
                        skip_group_check=True)
                rec = recp.tile([128, 1], f32, tag="rec")
                nc.vector.reciprocal(rec[:], og[:, 64:65])
                nc.vector.tensor_scalar_mul(
                    onorms[qt][:, h * 64:(h + 1) * 64], og[:, 0:64],
                    rec[:, 0:1])

            fillers = [lambda t=t: vproj_chunk(t) for t in range(16)]
            fillers += [lambda s=s: qk2_chunk(s) for s in range(4)]

            prev = None          # (pts, h, onorms) of the lagging head
            for qp in range(2):
                onorms = [onp_.tile([128, 192], f16, tag="onrm", name=f"on{qp}")
                          for _ in range(8)]
                for h in range(3):
                    pts = []
                    for kt in range(16):
                        if prev is not None and kt % 2 == 1:
                            av_qtile(prev[0], prev[1], (kt - 1) // 2, prev[2])
                        if fillers:
                            fillers.pop(0)()
                        pts.append(s_strip(qp, h, kt))
                        if prev is not None and prev[1] == 2 and kt == 15:
                            # prev half fully normalized -> queue its output
                            ponorms = prev[2]
                            fillers.extend(
                                [lambda q=q, po=ponorms:
                                 transpose_qt(qp - 1, q, po[q])
                                 for q in range(8)])
                            fillers.extend(
                                [lambda tb=tb, dg=dg:
                                 outproj_chunk(qp - 1, tb, dg)
                                 for tb in range(2) for dg in range(6)])
                    prev = (pts, h, onorms)

            # ------------- tail: last head's AV + norm + out ----------------
            for qt in range(8):
                av_qtile(prev[0], prev[1], qt, prev[2])
                if fillers:
                    fillers.pop(0)()
            for f in fillers:
                f()
            for q in range(8):
                transpose_qt(1, q, prev[2][q])
            for tb in range(2):
                for dg in range(6):
                    outproj_chunk(1, tb, dg)

            if dbg:
                def dump(dst, ap, pdim, free, n):
                    nchunk = (free + 511) // 512
                    for i in range(nchunk):
                        w = min(512, free - i * 512)
                        tf = stg.tile([128, 512], f32, tag="ot",
                                      name=f"dbg{n}_{i}")
                        nc.vector.tensor_copy(tf[0:pdim, 0:w],
                                              ap[:, i * 512:i * 512 + w])
                        nc.sync.dma_start(dst[:, i * 512:i * 512 + w],
                                          tf[0:pdim, 0:w])
                dump(gT_o, gT[:], 128, 48, 0)
                dump(qkT_o, qkT.rearrange("p a n -> p (a n)"), 128, 2 * N, 1)
                dump(qk2_o, qk2.rearrange("p a n -> p (a n)"), 64, 2 * N, 2)
                dump(vaug_o, vaug.rearrange("p a n -> p (a n)"), 128, 16 * 195, 3)
                dump(OTA_o, OTA[:], 128, N, 4)
                dump(OTB_o, OTB[:], 64, N, 5)

    nc.compile()
    return nc


def _in_maps(x, coverage, Wqkv, Wo, bo, Wg1, bg1, Wg2, bg2):
    x = np.asarray(x, np.float32)
    coverage = np.asarray(coverage, np.float32)
    Wqkv = np.asarray(Wqkv, np.float32)
    Wo = np.asarray(Wo, np.float32)
    Wg1 = np.asarray(Wg1, np.float32)
    bg1 = np.asarray(bg1, np.float32)
    Wg2 = np.asarray(Wg2, np.float32)
    bg2 = np.asarray(bg2, np.float32)
    q_all = Wqkv[:, 0:D] * np.float32(SCALE)
    k_all = Wqkv[:, D:2 * D]
    v_all = Wqkv[:, 2 * D:3 * D]
    bf = ml_dtypes.bfloat16
    maps = []
    for c in range(NCORES):
        b = c // 4
        hs = [3 * (c % 4) + j for j in range(3)]
        q = [q_all[:, h * HD:(h + 1) * HD] for h in hs]
        k = [k_all[:, h * HD:(h + 1) * HD] for h in hs]
        v = [v_all[:, h * HD:(h + 1) * HD] for h in hs]
        wqk = np.concatenate([q[0], q[1], k[0], k[1], q[2], k[2]], axis=1)
        wv = np.concatenate(v, axis=1)
        wo = np.concatenate([Wo[h * HD:(h + 1) * HD, :] for h in hs], axis=0)
        wg2_3 = np.stack([Wg2[:, h] for h in hs], axis=1)  # [192, 3]
        bg2_3 = np.array([[bg2[h] for h in hs]], np.float32)  # [1, 3]
        maps.append({
            "xT": np.ascontiguousarray(x[b].T).astype(bf),
            "cov": np.ascontiguousarray(coverage[b, :, 0].reshape(1, N)),
            "wqk": np.ascontiguousarray(wqk).astype(bf),
            "wv": np.ascontiguousarray(wv).astype(bf),
            "woA": np.ascontiguousarray(wo[0:128, :]).astype(np.float16),
            "woB": np.ascontiguousarray(wo[128:192, :]).astype(np.float16),
            "wg1": np.ascontiguousarray(Wg1),
            "bg1": np.ascontiguousarray(bg1.reshape(GATE_HID, 1)),
            "wg2a": np.ascontiguousarray(wg2_3[0:128, :]).astype(bf),
            "wg2b": np.ascontiguousarray(wg2_3[128:192, :]).astype(bf),
            "bg2r": np.ascontiguousarray(bg2_3).astype(bf),
        })
    return maps


def kernel_with_info(inputs, trace=False):
    from concourse.bass_utils import run_bass_kernel_spmd
    if "nc" not in _CACHE:
        _CACHE["nc"] = _build()
    nc = _CACHE["nc"]
    maps = _in_maps(**inputs)
    res = run_bass_kernel_spmd(nc, maps, list(range(NCORES)), trace=trace)
    bo = np.asarray(inputs["bo"], np.float32)
    out = np.empty((B, N, D), np.float32)
    for b in range(B):
        acc = np.zeros((D, N), np.float64)
        for c in range(4 * b, 4 * b + 4):
            acc += res.results[c]["outp"]
        out[b] = acc.T + bo
    return out, res


def kernel(**inputs):
    out, _ = kernel_with_info(inputs, trace=False)
    return out



# revision 20
# speedup vs baseline: 1.3668x; 1.3668x over previous
"""CoverageAwareAttention on 8 TRN2 NeuronCores — v4.

Structure vs v3:
- exp split across TWO engines: ACT (true exp, per-partition gate scale) and
  DVE (Schraudolph fast-exp: uint16(g*log2e*1024*s + B) bitcast to f16 — one
  tensor_scalar op per strip). Ratio tunable via DVE_FRAC.
- AV normalization moved to ACT (Relu activation with per-partition scale =
  reciprocal of the softmax denominator; og is strictly positive).
- batched input DMAs (xT in 4 token-slices), spread across SP/scalar/Pool
  issue queues; output DMAs f16 issued from gpsimd (SWDGE, idle engine).
- PE warmup matmuls at t=0 so the p-state is hot when real work arrives.
- strips start after only 3 qk chunks (cg0 s0/s1, cg1 s0); remaining qk
  chunks, qk2, vproj, AV, transpose, outproj all interleave as PE filler.
- one-head-lag pipeline retained from v3.
"""

import numpy as np
import ml_dtypes

B, N, D = 2, 2048, 768
H, HD = 12, 64
GATE_HID = D // 4
SCALE = HD ** -0.5
NCORES = 8
EXP_BIAS = -3.0
LOG2E = 1.4426950408889634
SCHRAU_SIGMA = 44.0
DVE_FRAC = 0.40625      # fraction of exp strips on DVE (Schraudolph)
OT_ACT_FRAC = 0.58      # fraction of output copies on ACT (rest DVE)

_CACHE = {}


def _build(dbg=False):
    import concourse.tile as tile
    import concourse.mybir as mybir
    from concourse import bacc
    from concourse.masks import make_identity

    f32 = mybir.dt.float32
    f32r = mybir.dt.float32r
    f16 = mybir.dt.float16
    bf16 = mybir.dt.bfloat16
    u16 = mybir.dt.uint16
    AF = mybir.ActivationFunctionType
    Alu = mybir.AluOpType

    nc = bacc.Bacc("TRN2", target_bir_lowering=False, debug=False,
                   num_devices=NCORES)

    xT = nc.dram_tensor("xT", [D, N], bf16, kind="ExternalInput").ap()
    covwg1 = nc.dram_tensor("covwg1", [1, N + GATE_HID], f32,
                            kind="ExternalInput").ap()
    wqk = nc.dram_tensor("wqk", [D, 384], bf16, kind="ExternalInput").ap()
    wv = nc.dram_tensor("wv", [D, 192], bf16, kind="ExternalInput").ap()
    woA = nc.dram_tensor("woA", [128, D], f16, kind="ExternalInput").ap()
    woB = nc.dram_tensor("woB", [64, D], f16, kind="ExternalInput").ap()
    bg1e = nc.dram_tensor("bg1e", [128, 2], f32, kind="ExternalInput").ap()
    wg2e = nc.dram_tensor("wg2e", [128, 9], bf16, kind="ExternalInput").ap()
    outp = nc.dram_tensor("outp", [D, N], f16, kind="ExternalOutput").ap()
    if dbg:
        gT_o = nc.dram_tensor("gT_o", [128, 48], f32, kind="ExternalOutput").ap()
        qkT_o = nc.dram_tensor("qkT_o", [128, 2 * N], f32, kind="ExternalOutput").ap()
        qk2_o = nc.dram_tensor("qk2_o", [64, 2 * N], f32, kind="ExternalOutput").ap()
        vaug_o = nc.dram_tensor("vaug_o", [128, 16 * 195], f32, kind="ExternalOutput").ap()
        OTA_o = nc.dram_tensor("OTA_o", [128, N], f32, kind="ExternalOutput").ap()
        OTB_o = nc.dram_tensor("OTB_o", [64, N], f32, kind="ExternalOutput").ap()
        PT0_o = nc.dram_tensor("PT0_o", [128, 1024], f32, kind="ExternalOutput").ap()
        PT1_o = nc.dram_tensor("PT1_o", [128, 1024], f32, kind="ExternalOutput").ap()
        on0_o = nc.dram_tensor("on0_o", [128, 192], f32, kind="ExternalOutput").ap()

    with tile.TileContext(nc) as tc:
        with tc.tile_pool(name="const", bufs=1) as cp, \
             tc.tile_pool(name="onrm", bufs=18) as onp_, \
             tc.tile_pool(name="recp", bufs=4) as recp, \
             tc.tile_pool(name="stg", bufs=8) as stg, \
             tc.tile_pool(name="ptp", bufs=36) as ptp, \
             tc.tile_pool(name="psS", bufs=3, space="PSUM") as psS, \
             tc.tile_pool(name="psD", bufs=2, space="PSUM") as psD:

            # ------------- PE warmup (keep p-state hot) ---------------------
            wtile = cp.tile([128, 512], f16, tag="wtile")
            nc.vector.memset(wtile[:], 0.0)
            for wi in range(6):
                pw = psD.tile([128, 512], f32, tag="D", name=f"warm{wi}")
                nc.tensor.matmul(pw[:], wtile[:, 0:128], wtile[:],
                                 start=True, stop=True, skip_group_check=True)
            # dummy silu: pulls the silu_and_others table load off the
            # critical path (overlaps the input-DMA wait)
            dumm = cp.tile([128, 1], f32, tag="dumm")
            nc.scalar.activation(dumm[:], wtile[:, 0:1], AF.Silu)

            # ------------- input DMAs ---------------------------------------
            # gate inputs packed: covwg1=[cov|wg1] f32, bg1e=[bg1a|bg1b] f32,
            # wg2e=[wg2a|wg2b|bg2] bf16 -> 3 DMAs on otherwise-idle queues
            cov_ext = cp.tile([1, N + GATE_HID], f32r, tag="cove")
            nc.scalar.dma_start(cov_ext[:], covwg1.bitcast(f32r))
            cov_sb = cov_ext[:, 0:N]
            wg1_sb = cov_ext[:, N:N + GATE_HID]
            bg1e_sb = cp.tile([128, 2], f32, tag="bg1e")
            nc.scalar.dma_start(bg1e_sb[:], bg1e)
            bg1a = bg1e_sb[:, 0:1]
            bg1b = bg1e_sb[0:64, 1:2]
            wg2e_sb = cp.tile([128, 9], bf16, tag="wg2e")
            nc.scalar.dma_start(wg2e_sb[:], wg2e)
            wg2a_sb = wg2e_sb[:, 0:3]
            wg2b_sb = wg2e_sb[0:64, 3:6]
            bg2_sb = wg2e_sb[0:1, 6:9]

            # big tensors via SP HWDGE, xT in 4 token-slices (wqk first)
            wqk_sb = cp.tile([128, 6, 384], bf16, tag="wqk")
            wqk_r = wqk.rearrange("(c p) f -> p c f", p=128)
            nc.sync.dma_start(wqk_sb[:], wqk_r)
            xT_sb = cp.tile([128, 6, N], bf16, tag="xT")
            xT_r = xT.rearrange("(c p) n -> p c n", p=128)
            for s in range(4):
                sl = slice(s * 512, (s + 1) * 512)
                nc.sync.dma_start(xT_sb[:, :, sl], xT_r[:, :, sl])
            wv_sb = cp.tile([128, 6, 192], bf16, tag="wv")
            wv_r = wv.rearrange("(c p) f -> p c f", p=128)
            nc.sync.dma_start(wv_sb[:], wv_r)
            woA_sb = cp.tile([128, D], f16, tag="woA")
            nc.sync.dma_start(woA_sb[:], woA)
            woB_sb = cp.tile([64, D], f16, tag="woB")
            nc.sync.dma_start(woB_sb[:], woB)

            # ------------- const tiles --------------------------------------
            identF = cp.tile([128, 128], f16, tag="identF")
            make_identity(nc, identF[:])
            ebias = cp.tile([128, 1], f32, tag="ebias")
            nc.vector.memset(ebias[:], EXP_BIAS)
            ones1 = cp.tile([1, 128], bf16, tag="ones1")
            nc.vector.memset(ones1[:], 1.0)
            vaug = cp.tile([128, 16, 195], f16, tag="vaug")
            nc.vector.memset(
                vaug.rearrange("p t (h c) -> p t h c", c=65)[:, :, :, 64], 1.0)

            qkT = cp.tile([128, 2, N], f16, tag="qkT")
            qk2 = cp.tile([64, 2, N], f16, tag="qk2")
            gT = cp.tile([128, 48], f32, tag="gT")
            gS = cp.tile([128, 48], f32, tag="gS")
            ga_sb = cp.tile([128, 4, 512], bf16, tag="ga")
            gb_sb = cp.tile([64, 4, 512], bf16, tag="gb")
            OTA = cp.tile([128, N], f16, tag="OTA")
            OTB = cp.tile([64, N], f16, tag="OTB")

            # ------------- gate: hidden = silu(cov @ Wg1 + bg1) -------------
            for half in range(2):
                pa = psS.tile([128, 1024], f32, tag="S")
                for si in range(2):
                    sl = slice((2 * half + si) * 512, (2 * half + si + 1) * 512)
                    nc.tensor.matmul(pa[:, si * 512:(si + 1) * 512],
                                     wg1_sb[0:1, 0:128], cov_sb[0:1, sl],
                                     start=True, stop=True, skip_group_check=True)
                nc.scalar.activation(ga_sb[:, 2 * half:2 * half + 2, :], pa[:],
                                     AF.Silu, bias=bg1a)
            for half in range(2):
                pb = psS.tile([64, 1024], f32, tag="S")
                for si in range(2):
                    sl = slice((2 * half + si) * 512, (2 * half + si + 1) * 512)
                    nc.tensor.matmul(pb[:, si * 512:(si + 1) * 512],
                                     wg1_sb[0:1, 128:192], cov_sb[0:1, sl],
                                     start=True, stop=True, skip_group_check=True)
                nc.scalar.activation(gb_sb[:, 2 * half:2 * half + 2, :], pb[:],
                                     AF.Silu, bias=bg1b)

            # ------------- gate: zT[k, h] = hidden^T @ Wg2 + bg2 ------------
            # sigmoid(z) = 0.5*tanh(z/2) + 0.5 -- tanh shares the silu table
            # set, so no extra ACT table load on the critical path
            zT_ps = psS.tile([128, 48], f32, tag="S")
            for kt in range(16):
                si, j = kt // 4, kt % 4
                out = zT_ps[:, 3 * kt:3 * kt + 3]
                nc.tensor.matmul(out, ga_sb[:, si, j * 128:(j + 1) * 128],
                                 wg2a_sb, start=True, stop=False,
                                 skip_group_check=True)
                nc.tensor.matmul(out, gb_sb[:, si, j * 128:(j + 1) * 128],
                                 wg2b_sb, start=False, stop=False,
                                 skip_group_check=True)
                nc.tensor.matmul(out, ones1[0:1, :], bg2_sb,
                                 start=False, stop=True,
                                 skip_group_check=True)
            gth = cp.tile([128, 48], f32, tag="gth")
            nc.scalar.activation(gth[:], zT_ps[:], AF.Tanh, scale=0.5)
            nc.vector.tensor_scalar(gT[:], gth[:], 0.5, 0.5, Alu.mult, Alu.add)
            nc.vector.tensor_scalar(gS[:], gth[:], 512.0 * LOG2E,
                                    512.0 * LOG2E, Alu.mult, Alu.add)
            B_SCHRAU = 1024.0 * (15.0 + EXP_BIAS * LOG2E) - SCHRAU_SIGMA
            # dummy exp pulls the exp_and_others table load ahead of strips
            # (reads gth so it cannot be scheduled before the silu/tanh block)
            nc.scalar.activation(dumm[:], gth[:, 0:1], AF.Exp)

            # ------------- qk projection chunks ------------------------------
            def qk_chunk(cg, s):
                ps = psD.tile([128, 512], f32, tag="D")
                for c in range(6):
                    nc.tensor.matmul(ps[:],
                                     wqk_sb[:, c, cg * 128:(cg + 1) * 128],
                                     xT_sb[:, c, s * 512:(s + 1) * 512],
                                     start=(c == 0), stop=(c == 5),
                                     skip_group_check=True)
                nc.vector.tensor_copy(qkT[:, cg, s * 512:(s + 1) * 512], ps[:])

            # strips for (qp=0, h=0/1) need: cg0 s0, cg0 s1 (q), cg1 kt-tiles
            qk_chunk(0, 0)
            qk_chunk(0, 1)
            qk_chunk(1, 0)

            # ------------- filler helpers -----------------------------------
            def vproj_chunk(t):
                ps = psD.tile([128, 192], f32, tag="D",
                              padded_shape=[128, 512])
                for c in range(6):
                    nc.tensor.matmul(ps[:], xT_sb[:, c, t * 128:(t + 1) * 128],
                                     wv_sb[:, c, :], start=(c == 0),
                                     stop=(c == 5), skip_group_check=True)
                nc.vector.tensor_copy(
                    vaug.rearrange("p t (h c) -> p t h c", c=65)
                    [:, t, :, 0:64],
                    ps.rearrange("p (h c) -> p h c", c=64)[:])

            def qk2_chunk(s):
                ps = psD.tile([128, 512], f32, tag="D")
                for c in range(6):
                    nc.tensor.matmul(ps[:], wqk_sb[:, c, 256:384],
                                     xT_sb[:, c, s * 512:(s + 1) * 512],
                                     start=(c == 0), stop=(c == 5),
                                     skip_group_check=True)
                sl = slice(s * 512, (s + 1) * 512)
                nc.vector.tensor_copy(qk2[:, 0, sl], ps[0:64, :])
                nc.vector.tensor_copy(qk2[:, 1, sl], ps[64:128, :])

            def transpose_qt(qp, qt, onorm, pool=None):
                qg = qp * 8 + qt
                T = (pool or psD).tile([128, 256], f16,
                                       tag="S" if pool is not None else "D")
                nc.tensor.transpose(T[:, 0:128], onorm[:, 0:128], identF[:])
                nc.tensor.transpose(T[0:64, 128:256], onorm[:, 128:192],
                                    identF[:])
                nc.vector.tensor_copy(OTA[:, qg * 128:(qg + 1) * 128],
                                      T[:, 0:128])
                nc.vector.tensor_copy(OTB[:, qg * 128:(qg + 1) * 128],
                                      T[0:64, 128:256])

            ot_acc = [0.0]

            def outproj_chunk(qp, tb, dg, pool=None):
                t0 = qp * 1024 + tb * 512
                ps = (pool or psD).tile([128, 512], f32,
                                        tag="S" if pool is not None else "D")
                nc.tensor.matmul(ps[:], woA_sb[:, dg * 128:(dg + 1) * 128],
                                 OTA[:, t0:t0 + 512], start=True, stop=False,
                                 skip_group_check=True)
                nc.tensor.matmul(ps[:], woB_sb[:, dg * 128:(dg + 1) * 128],
                                 OTB[:, t0:t0 + 512], start=False, stop=True,
                                 skip_group_check=True)
                ot = stg.tile([128, 512], f16, tag="ot")
                ot_acc[0] += OT_ACT_FRAC
                if ot_acc[0] >= 1.0:
                    ot_acc[0] -= 1.0
                    nc.scalar.activation(ot[:], ps[:], AF.Copy)
                else:
                    nc.vector.tensor_copy(ot[:], ps[:])
                eng = nc.gpsimd if (qp == 0 or (tb == 1 and dg % 2)) \
                    else nc.sync
                eng.dma_start(outp[dg * 128:(dg + 1) * 128, t0:t0 + 512],
                              ot[:])

            # ------------- attention ----------------------------------------
            dve_acc = [0.0]

            def s_strip(qp, h, kt):
                S_ps = psS.tile([128, 1024], f32, tag="S")
                ksl = slice(kt * 128, (kt + 1) * 128)
                for qs in range(2):
                    q0 = qp * 1024 + qs * 512
                    out = S_ps[:, qs * 512:(qs + 1) * 512]
                    if h == 0:
                        nc.tensor.matmul(out, qkT[0:64, 1, ksl],
                                         qkT[0:64, 0, q0:q0 + 512],
                                         start=True, stop=True,
                                         skip_group_check=True)
                    elif h == 1:
                        nc.tensor.matmul(out, qkT[64:128, 1, ksl],
                                         qkT[64:128, 0, q0:q0 + 512],
                                         start=True, stop=True,
                                         skip_group_check=True)
                    else:
                        nc.tensor.matmul(out, qk2[:, 1, ksl],
                                         qk2[:, 0, q0:q0 + 512],
                                         start=True, stop=True,
                                         skip_group_check=True)
                PT = ptp.tile([128, 1024], f16, tag="PT")
                col = slice(3 * kt + h, 3 * kt + h + 1)
                dve_acc[0] += DVE_FRAC
                if dve_acc[0] >= 1.0:
                    dve_acc[0] -= 1.0
                    nc.vector.tensor_scalar(PT.bitcast(u16)[:], S_ps[:],
                                            gS[:, col], B_SCHRAU,
                                            Alu.mult, Alu.add)
                else:
                    nc.scalar.activation(PT[:], S_ps[:], AF.Exp,
                                         bias=ebias[:, 0:1],
                                         scale=gT[:, col])
                return PT

            def av_qtile(pts, h, qt, onorms, pool=None):
                og = (pool or psD).tile([128, 65], f32,
                                        tag="S" if pool is not None else "D",
                                        padded_shape=[128, 512])
                for kt in range(16):
                    nc.tensor.matmul(
                        og[:], pts[kt][:, qt * 128:(qt + 1) * 128],
                        vaug[:, kt, h * 65:h * 65 + 65],
                        start=(kt == 0), stop=(kt == 15),
                        skip_group_check=True)
                rec = recp.tile([128, 1], f32, tag="rec")
                nc.vector.reciprocal(rec[:], og[:, 64:65])
                nc.vector.tensor_scalar_mul(
                    onorms[qt][:, h * 64:(h + 1) * 64], og[:, 0:64],
                    rec[:, 0:1])

            fillers = [lambda s=s: qk_chunk(1, s) for s in (1, 2, 3)]
            fillers += [lambda s=s: qk_chunk(0, s) for s in (2, 3)]
            fillers += [lambda s=s: qk2_chunk(s) for s in range(4)]
            fillers += [lambda t=t: vproj_chunk(t) for t in range(16)]

            prev = None          # (pts, h, onorms) of the lagging head
            dbg_keep = {}
            for qp in range(2):
                onorms = [onp_.tile([128, 192], f16, tag="onrm", name=f"on{qp}")
                          for _ in range(8)]
                for h in range(3):
                    pts = []
                    for kt in range(16):
                        if prev is not None and kt % 2 == 1:
                            av_qtile(prev[0], prev[1], (kt - 1) // 2, prev[2])
                        if fillers:
                            fillers.pop(0)()
                        if fillers and qp == 0 and h == 0:
                            fillers.pop(0)()
                        pts.append(s_strip(qp, h, kt))
                        if dbg and qp == 0 and h == 0 and kt in (0, 1):
                            dbg_keep[f"PT{kt}"] = pts[kt]
                        if prev is not None and prev[1] == 2 and kt == 15:
                            # prev half fully normalized -> queue its output
                            ponorms = prev[2]
                            fillers.extend(
                                [lambda q=q, po=ponorms:
                                 transpose_qt(qp - 1, q, po[q])
                                 for q in range(8)])
                            fillers.extend(
                                [lambda tb=tb, dg=dg:
                                 outproj_chunk(qp - 1, tb, dg)
                                 for tb in range(2) for dg in range(6)])
                    prev = (pts, h, onorms)
                if dbg and qp == 0:
                    dbg_keep["on0"] = onorms[0]

            # ------------- tail: last head's AV + norm + out ----------------
            # strips are done -> psS banks are free; use them to widen the
            # og/outproj pipeline. Interleave transpose right after each AV
            # and start outproj waves as soon as their token range is ready.
            for qt in range(8):
                av_qtile(prev[0], prev[1], qt, prev[2],
                         pool=(psS if qt % 2 else None))
                if fillers:
                    fillers.pop(0)()
                transpose_qt(1, qt, prev[2][qt],
                             pool=(psS if qt % 2 == 0 else None))
                if qt == 4:
                    for dg in range(3):
                        outproj_chunk(1, 0, dg, pool=(psS if dg % 2 else None))
            for f in fillers:
                f()
            for dg in range(3, 6):
                outproj_chunk(1, 0, dg, pool=(psS if dg % 2 else None))
            for dg in range(6):
                outproj_chunk(1, 1, dg, pool=(psS if dg % 2 else None))

            if dbg:
                def dump(dst, ap, pdim, free, n):
                    nchunk = (free + 511) // 512
                    for i in range(nchunk):
                        w = min(512, free - i * 512)
                        tf = stg.tile([128, 512], f32, tag="dbg",
                                      name=f"dbg{n}_{i}")
                        nc.vector.tensor_copy(tf[0:pdim, 0:w],
                                              ap[:, i * 512:i * 512 + w])
                        nc.sync.dma_start(dst[:, i * 512:i * 512 + w],
                                          tf[0:pdim, 0:w])
                dump(gT_o, gT[:], 128, 48, 0)
                dump(qkT_o, qkT.rearrange("p a n -> p (a n)"), 128, 2 * N, 1)
                dump(qk2_o, qk2.rearrange("p a n -> p (a n)"), 64, 2 * N, 2)
                dump(vaug_o, vaug.rearrange("p a n -> p (a n)"), 128, 16 * 195, 3)
                dump(OTA_o, OTA[:], 128, N, 4)
                dump(OTB_o, OTB[:], 64, N, 5)
                dump(PT0_o, dbg_keep["PT0"][:], 128, 1024, 6)
                dump(PT1_o, dbg_keep["PT1"][:], 128, 1024, 7)
                dump(on0_o, dbg_keep["on0"][:], 128, 192, 8)

    nc.compile()
    return nc


def _in_maps(x, coverage, Wqkv, Wo, bo, Wg1, bg1, Wg2, bg2):
    x = np.asarray(x, np.float32)
    coverage = np.asarray(coverage, np.float32)
    Wqkv = np.asarray(Wqkv, np.float32)
    Wo = np.asarray(Wo, np.float32)
    Wg1 = np.asarray(Wg1, np.float32)
    bg1 = np.asarray(bg1, np.float32)
    Wg2 = np.asarray(Wg2, np.float32)
    bg2 = np.asarray(bg2, np.float32)
    q_all = Wqkv[:, 0:D] * np.float32(SCALE)
    k_all = Wqkv[:, D:2 * D]
    v_all = Wqkv[:, 2 * D:3 * D]
    bf = ml_dtypes.bfloat16
    maps = []
    for c in range(NCORES):
        b = c // 4
        hs = [3 * (c % 4) + j for j in range(3)]
        q = [q_all[:, h * HD:(h + 1) * HD] for h in hs]
        k = [k_all[:, h * HD:(h + 1) * HD] for h in hs]
        v = [v_all[:, h * HD:(h + 1) * HD] for h in hs]
        wqk = np.concatenate([q[0], q[1], k[0], k[1], q[2], k[2]], axis=1)
        wv = np.concatenate(v, axis=1)
        wo = np.concatenate([Wo[h * HD:(h + 1) * HD, :] for h in hs], axis=0)
        wg2_3 = np.stack([Wg2[:, h] for h in hs], axis=1)  # [192, 3]
        bg2_3 = np.array([[bg2[h] for h in hs]], np.float32)  # [1, 3]
        covwg1 = np.concatenate(
            [coverage[b, :, 0].reshape(1, N), Wg1.reshape(1, GATE_HID)],
            axis=1).astype(np.float32)
        bg1e = np.zeros((128, 2), np.float32)
        bg1e[:, 0] = bg1[0:128]
        bg1e[0:64, 1] = bg1[128:192]
        wg2e = np.zeros((128, 9), np.float32)
        wg2e[:, 0:3] = wg2_3[0:128, :]
        wg2e[0:64, 3:6] = wg2_3[128:192, :]
        wg2e[0:1, 6:9] = bg2_3
        maps.append({
            "xT": np.ascontiguousarray(x[b].T).astype(bf),
            "covwg1": np.ascontiguousarray(covwg1),
            "wqk": np.ascontiguousarray(wqk).astype(bf),
            "wv": np.ascontiguousarray(wv).astype(bf),
            "woA": np.ascontiguousarray(wo[0:128, :]).astype(np.float16),
            "woB": np.ascontiguousarray(wo[128:192, :]).astype(np.float16),
            "bg1e": np.ascontiguousarray(bg1e),
            "wg2e": np.ascontiguousarray(wg2e).astype(bf),
        })
    return maps


def kernel_with_info(inputs, trace=False):
    from concourse.bass_utils import run_bass_kernel_spmd
    if "nc" not in _CACHE:
        _CACHE["nc"] = _build()
    nc = _CACHE["nc"]
    maps = _in_maps(**inputs)
    res = run_bass_kernel_spmd(nc, maps, list(range(NCORES)), trace=trace)
    bo = np.asarray(inputs["bo"], np.float32)
    out = np.empty((B, N, D), np.float32)
    for b in range(B):
        acc = np.zeros((D, N), np.float64)
        for c in range(4 * b, 4 * b + 4):
            acc += res.results[c]["outp"].astype(np.float64)
        out[b] = acc.T + bo
    return out, res


def kernel(**inputs):
    out, _ = kernel_with_info(inputs, trace=False)
    return out
